# revision 16
# baseline (speedup 1.0000x reference)
"""Baichuan attention (ALiBi + causal) on 8 TRN2 NeuronCores.

Sharding: tensor-parallel over heads, 5 per core, dealt by ALiBi-slope rank
(core c takes slope-ranks {8*slot + c}) so that every core's head-slot hh has
a similar attention span; o_proj partials are summed on host.

Precision: compensated fp8 (hi+lo e4m3, per-tensor/per-head pow2 scaling) with
DoubleRow perf mode for the QKV projection and o_proj; fp16 for QK/PV;
ALiBi-decay tile skipping per head-slot.

Schedule: software-pipelined across chunks —
  phase A: QKV(chunk 0) with DMA-paced startup
  phase B: QKV(chunk c) with attention(chunk c-1) interleaved at pass
           boundaries (softmax chains hide behind dense QKV matmuls)
  phase C: attention(chunk 3) interleaved with o_proj(chunk 0), then
           o_proj(chunks 1-3); o_proj weights streamed at phase start.

All shapes hardcoded for: B=1, S=2048, H=5120, nh=40, hd=128.
"""

import math
from contextlib import ExitStack

import numpy as np
import ml_dtypes

import concourse.bass as bass
import concourse.bacc as bacc
import concourse.mybir as mybir
import concourse.tile as tile
from concourse.bass_utils import run_bass_kernel_spmd

E4 = mybir.dt.float8e4
F16 = mybir.dt.float16
F32 = mybir.dt.float32
NP_E4 = ml_dtypes.float8_e4m3

NH = 40
HD = 128
H = NH * HD          # 5120
S = 2048
NCORES = 8
HPC = NH // NCORES   # heads per core = 5
OPC = HPC * HD       # output features per core = 640

S_CHUNK = 512
N_SCHUNK = S // S_CHUNK          # 4
NK = H // 128                    # 40 k-tiles (contraction for QKV)
N_ST = S // 128                  # 16 s-tiles
WBLK = 4                         # k-tiles per weight DMA block
NBLK = NK // WBLK                # 10
HID_HALF = NK // 2               # 20 k-tiles per hid half-tile
C_CTX = 16.0                     # fixed on-chip scale for ctx fp8 split
DR = mybir.MatmulPerfMode.DoubleRow
D_CHUNK = 512                    # score-row chunk (1 PSUM bank)
PTC_CAP = 14                     # max sk-tiles per pTc segment
# ALiBi prune: keep sk-tile j for sq-tile t iff slope*(128*(t-j)) < TILE_D.
# Worst-case dropped-mass exponent is TILE_D - slope*127 (first row of the
# query tile), so TILE_D=9 bounds leaked softmax mass at ~e^-7 per head.
TILE_D = 9.0


def _alibi_slopes(n: int):
    def pow2_slopes(k):
        start = 2.0 ** (-(2.0 ** -(math.log2(k) - 3)))
        return [start * (start ** i) for i in range(k)]
    if math.log2(n).is_integer():
        return pow2_slopes(n)
    closest = 2 ** int(math.floor(math.log2(n)))
    return pow2_slopes(closest) + _alibi_slopes(2 * closest)[0::2][: n - closest]


_SLOPES = np.array(_alibi_slopes(NH), np.float64)
# head dealing: rank by slope desc; core c, slot hh -> head _ORDER[8*hh + c]
_ORDER = np.argsort(-_SLOPES, kind="stable")
# per-slot causal+ALiBi tile cap: keep sk-tile j for sq-tile t iff t - j < tcap
_TCAP = []
for hh in range(HPC):
    smin = _SLOPES[_ORDER[8 * hh:8 * hh + 8]].min()
    D = TILE_D / smin
    _TCAP.append(min(N_ST, int(math.floor((D + 127.0) / 128.0)) + 1))


def _jlo(t, hh):
    return max(0, t - _TCAP[hh] + 1)


def build_nc() -> bass.Bass:
    nc = bacc.Bacc(None)
    marks = {}

    def _mark(phase):
        import re as _re
        mx = 0
        for _n in nc.inst_map:
            m = _re.match(r'I-(\d+)$', _n)
            if m:
                mx = max(mx, int(m.group(1)))
        marks[phase] = mx + 1

    # hid[sc, p, k, i, s]: i=0 hi, i=1 lo of C_hs*hidden[sc*512+s, k*128+p]
    hid_d = nc.declare_dram_parameter(
        "hid", [N_SCHUNK, 128, NK, 2, S_CHUNK], E4, isOutput=False)
    # w*[blk, p, j, i, o]: i=0 lo, i=1 hi of scaled w.T[blk*512+j*128+p, o]
    wq_d = nc.declare_dram_parameter("wq", [NBLK, 128, WBLK, 2, OPC], E4, isOutput=False)
    wk_d = nc.declare_dram_parameter("wk", [NBLK, 128, WBLK, 2, OPC], E4, isOutput=False)
    wv_d = nc.declare_dram_parameter("wv", [NBLK, 128, WBLK, 2, OPC], E4, isOutput=False)
    # wo[nk, p, slot, i, o]: slots 0..4 = d-tile h with (lo,hi); slot 5 = d-tile 4 (hi,lo)
    wo_d = nc.declare_dram_parameter("wo", [H // 512, 128, 6, 2, 512], E4, isOutput=False)
    kb_d = nc.declare_dram_parameter("kb", [128, S], F16, isOutput=False)
    qb_d = nc.declare_dram_parameter("qb", [128, HPC, N_ST], F32, isOutput=False)
    slopes_d = nc.declare_dram_parameter("slopes", [128, HPC], F32, isOutput=False)
    # dsc[p, 0..2, hh] = descale for q/k/v psum of head-slot hh; dsc[p, 3, 0] = o_proj descale
    dsc_d = nc.declare_dram_parameter("dsc", [128, 4, HPC], F32, isOutput=False)
    out_d = nc.declare_dram_parameter("out", [S, H], F16, isOutput=True)

    with ExitStack() as ctx:
        tc = ctx.enter_context(tile.TileContext(nc))

        # ---- persistent SBUF residents ----
        qkv_pool = ctx.enter_context(tc.tile_pool(name="qkv", bufs=1))

        qT = qkv_pool.tile([128, HPC, S], F16, tag="qT")     # qT[p, hh, s] = q[s, hh*128+p]
        kT = qkv_pool.tile([128, HPC, S], F16, tag="kT")
        vS = qkv_pool.tile([128, HPC, N_ST, 128], F16, tag="vS")  # vS[p, hh, j, d]
        # ctxc[p, hh, i, s]: i=0 hi, i=1 lo of C_CTX*ctx[s, hh*128+p]
        ctxc = qkv_pool.tile([128, HPC, 2, S], E4, tag="ctxc")
        dsc = qkv_pool.tile([128, 4, HPC], F32, tag="dsc")
        nc.sync.dma_start(dsc[:], dsc_d[:])
        kb = qkv_pool.tile([128, S], F16, tag="kb")
        qb = qkv_pool.tile([128, HPC, N_ST], F32, tag="qb")
        slopes_t = qkv_pool.tile([128, HPC], F32, tag="slopes_t")

        # ---- unified PSUM pool: one rotating ring of 8 banks shared by the
        # QKV passes (5 live), V tail tiles, scores, PV, and o_proj.  PSUM
        # accumulation groups are tracked per 2KB zero region (= bank), so
        # every tile gets its own bank; a single tag keeps allocation FIFO.
        psU = ctx.enter_context(tc.tile_pool(name="psU", bufs=8, space="PSUM"))
        pexp_pool = ctx.enter_context(tc.tile_pool(name="pexp", bufs=4))
        pnorm_pool = ctx.enter_context(tc.tile_pool(name="pnorm", bufs=3))
        ptmp_pool = ctx.enter_context(tc.tile_pool(name="ptmp", bufs=2))
        pT_pool = ctx.enter_context(tc.tile_pool(name="pTc", bufs=2))
        stats_pool = ctx.enter_context(tc.tile_pool(name="stats", bufs=8))

        pTcs = {}

        def emit_scores(C, hh):
            jmin = _jlo(4 * C, hh)             # lowest sk-tile used in chunk
            njc = 4 * C + 4 - jmin
            segs = []                          # (j_abs_start, width, tile)
            for s0 in range(0, njc, PTC_CAP):
                w = min(PTC_CAP, njc - s0)
                ptile = pT_pool.tile([128, w, 512], F16, tag="pTc",
                                     name=f"pTc_{C}_{hh}_{s0}",
                                     padded_shape=[128, PTC_CAP, 512])
                segs.append((jmin + s0, w, ptile))
            pTcs[(C, hh)] = segs

            def ptc_slot(j_abs):
                for js, w, ptile in segs:
                    if js <= j_abs < js + w:
                        return ptile, j_abs - js
                raise AssertionError(j_abs)

            for ti in range(4):
                t = 4 * C + ti
                j0 = _jlo(t, hh)
                L = 128 * (t + 1 - j0)         # score row width (cols j0*128..)
                k_base = j0 * 128
                nch = (L + D_CHUNK - 1) // D_CHUNK
                rs = stats_pool.tile([128, 4], F32, tag="rs")
                pexp_tiles = []
                for ci in range(nch):
                    W = min(D_CHUNK, L - ci * D_CHUNK)
                    ps = psU.tile([128, D_CHUNK], F32, tag="u", name="ps_s")
                    kc0 = k_base + ci * D_CHUNK
                    nc.tensor.matmul(
                        ps[:, 0:W],
                        lhsT=qT[:, hh, t * 128:(t + 1) * 128],
                        rhs=kT[:, hh, kc0:kc0 + W],
                        start=True, stop=True,
                    )
                    # + sk on DVE (the -sq part rides the exp bias); only DVE
                    # and Act can read PSUM, and this add gates psS release
                    tmp = ptmp_pool.tile([128, D_CHUNK], F32, tag="ptmp")
                    nc.vector.tensor_add(tmp[:, :W], ps[:, :W], kb[:, kc0:kc0 + W])
                    if ci == nch - 1:
                        # causal mask on the diagonal 128-block: keep j<=p
                        nc.gpsimd.affine_select(
                            tmp[:, W - 128:W], tmp[:, W - 128:W],
                            pattern=[[-1, 128]],
                            compare_op=mybir.AluOpType.is_ge,
                            fill=-1e9, base=0, channel_multiplier=1)
                    pe = pexp_pool.tile([128, D_CHUNK], F16, tag="pe")
                    nc.scalar.activation(
                        pe[:, :W], tmp[:, :W],
                        mybir.ActivationFunctionType.Exp,
                        scale=slopes_t[:, hh:hh + 1],
                        bias=qb[:, hh, t:t + 1],
                        accum_out=rs[:, ci:ci + 1],
                    )
                    pexp_tiles.append(pe)

                rcp = stats_pool.tile([128, 1], F32, tag="rcp")
                if nch > 1:
                    tot = stats_pool.tile([128, 1], F32, tag="tot")
                    nc.vector.reduce_sum(tot[:], rs[:, :nch], axis=mybir.AxisListType.X)
                    nc.vector.reciprocal(rcp[:], tot[:])
                else:
                    nc.vector.reciprocal(rcp[:], rs[:, 0:1])

                for ci in range(nch):
                    W = min(D_CHUNK, L - ci * D_CHUNK)
                    nb = W // 128
                    pn = pnorm_pool.tile([128, D_CHUNK], F16, tag="pn")
                    nc.vector.tensor_scalar_mul(pn[:, :W], pexp_tiles[ci][:, :W], rcp[:, 0:1])
                    for jj in range(nb):
                        j_abs = j0 + ci * (D_CHUNK // 128) + jj
                        ptile, slot = ptc_slot(j_abs)
                        nc.sync.dma_start_transpose(
                            out=ptile[:, slot, ti * 128:(ti + 1) * 128],
                            in_=pn[:, jj * 128:(jj + 1) * 128],
                        )

        def emit_pv(C, hh):
            segs = pTcs.pop((C, hh))

            def ptc_slot(j_abs):
                for js, w, ptile in segs:
                    if js <= j_abs < js + w:
                        return ptile, j_abs - js
                raise AssertionError(j_abs)

            pso = psU.tile([128, 512], F32, tag="u", name="ps_o")
            for ti in range(4):
                t = 4 * C + ti
                j0 = _jlo(t, hh)
                for j in range(j0, t + 1):
                    ptile, slot = ptc_slot(j)
                    nc.tensor.matmul(
                        pso[:, ti * 128:(ti + 1) * 128],
                        lhsT=vS[:, hh, j, :],
                        rhs=ptile[:, slot, ti * 128:(ti + 1) * 128],
                        start=(j == j0), stop=(j == t),
                    )
            # ctx -> fp8 hi/lo at scale C_CTX
            cblk = slice(C * 512, (C + 1) * 512)
            if (C * HPC + hh) % 2 == 0:
                nc.scalar.activation(
                    ctxc[:, hh, 0, cblk], pso[:],
                    mybir.ActivationFunctionType.Copy, scale=C_CTX)
            else:
                nc.vector.tensor_scalar_mul(
                    ctxc[:, hh, 0, cblk], pso[:], C_CTX)
            nc.vector.scalar_tensor_tensor(
                ctxc[:, hh, 1, cblk], pso[:], C_CTX, ctxc[:, hh, 0, cblk],
                op0=mybir.AluOpType.mult, op1=mybir.AluOpType.subtract)

        # ================= QKV + interleaved attention ======================
        with (
            tc.tile_pool(name="hid", bufs=3) as hid_pool,
            tc.tile_pool(name="wstream", bufs=4) as w_pool,
        ):
            # -- weight stream: global consumption order, prefetch ahead --
            worder = []
            for sc in range(N_SCHUNK):
                for w_d in (wq_d, wk_d, wv_d):
                    for blk in range(NBLK):
                        worder.append((w_d, blk))
            wstate = {"next": 1, "tiles": {}}   # idx 0 (chunk0 wq blk0) is manual

            def w_issue(n=1):
                for _ in range(n):
                    i = wstate["next"]
                    if i >= len(worder):
                        return
                    w_d, blk = worder[i]
                    t = w_pool.tile([128, WBLK, 2, OPC], E4, tag="wt", name=f"wt{i}")
                    nc.sync.dma_start(t[:], w_d[blk])
                    wstate["tiles"][i] = t
                    wstate["next"] = i + 1

            def w_issue_until(i):
                while wstate["next"] <= min(i, len(worder) - 1):
                    w_issue()

            def w_get(i):
                return wstate["tiles"].pop(i)

            # -- hid half-tiles --
            hid_tiles = {}

            def hid_tile(sc, half):
                key = (sc, half)
                if key not in hid_tiles:
                    hid_tiles[key] = hid_pool.tile(
                        [128, HID_HALF, 2, S_CHUNK], E4, tag="hid",
                        name=f"hid_{sc}_{half}")
                return hid_tiles[key]

            def hid_dma(sc, half, k0, k1):
                ht = hid_tile(sc, half)
                nc.sync.dma_start(ht[:, k0:k1],
                                  hid_d[sc, :, half * HID_HALF + k0:half * HID_HALF + k1])

            def ht_slices(sc, blk):
                """(tile, local k offset) for k-tiles blk*4..blk*4+4."""
                half, loc = divmod(blk * WBLK, HID_HALF)
                return hid_tile(sc, half), loc

            def qk_pass(sc, w_d, dest, di, wbase, first=False):
                """q or k pass: psum[oi] = [128 o, 512 s]."""
                pss = [psU.tile([128, S_CHUNK], F32, tag="u", name=f"ps{_i}")
                       for _i in range(HPC)]
                for blk in range(NBLK):
                    if first and blk == 0:
                        wt = wstate["tiles"].pop(-1)   # manual chunk-0 wq blk0
                    else:
                        wt = w_get(wbase + blk)
                    w_issue_until(wbase + blk + 3)
                    ht, loc = ht_slices(sc, blk)
                    for oi in range(HPC):
                        ocol = slice(oi * 128, (oi + 1) * 128)
                        for i in range(WBLK // 2):   # main: hi x hi pairs
                            nc.tensor.matmul(
                                pss[oi][:],
                                lhsT=wt[:, 2 * i:2 * i + 2, 1, ocol],
                                rhs=ht[:, loc + 2 * i:loc + 2 * i + 2, 0, :],
                                start=(blk == 0 and i == 0), stop=False,
                                perf_mode=DR,
                            )
                        for j in range(WBLK):        # cross: (lo,hi) x (hi,lo)
                            nc.tensor.matmul(
                                pss[oi][:],
                                lhsT=wt[:, j, :, ocol],
                                rhs=ht[:, loc + j, :, :],
                                start=False,
                                stop=(blk == NBLK - 1 and j == WBLK - 1),
                                perf_mode=DR,
                            )
                for oi in range(HPC):
                    dslice = dest[:, oi, sc * S_CHUNK:(sc + 1) * S_CHUNK]
                    if oi % 2 == 0:
                        nc.vector.tensor_scalar_mul(
                            dslice, pss[oi][:], dsc[:, di, oi:oi + 1])
                    else:
                        nc.scalar.activation(
                            dslice, pss[oi][:],
                            mybir.ActivationFunctionType.Copy,
                            scale=dsc[:, di, oi:oi + 1])

            def v_pass(sc, wbase):
                """v pass: per m-tile psum [128 s, 512 o] + packed 128-col tile."""
                vps = [psU.tile([128, S_CHUNK], F32, tag="u", name=f"vps0_{m}")
                       for m in range(4)]
                vps1 = [psU.tile([128, 128], F32, tag="u", name=f"vps1_{m}",
                                 padded_shape=[128, 512])
                        for m in range(4)]
                for blk in range(NBLK):
                    wt = w_get(wbase + blk)
                    w_issue_until(wbase + blk + 3)
                    ht, loc = ht_slices(sc, blk)
                    for m in range(4):
                        mrow = slice(m * 128, (m + 1) * 128)
                        for i in range(WBLK // 2):
                            kk = loc + 2 * i
                            st = (blk == 0 and i == 0)
                            nc.tensor.matmul(
                                vps[m][:],
                                lhsT=ht[:, kk:kk + 2, 0, mrow],
                                rhs=wt[:, 2 * i:2 * i + 2, 1, 0:512],
                                start=st, stop=False, perf_mode=DR,
                            )
                            nc.tensor.matmul(
                                vps1[m][:],
                                lhsT=ht[:, kk:kk + 2, 0, mrow],
                                rhs=wt[:, 2 * i:2 * i + 2, 1, 512:640],
                                start=st, stop=False, perf_mode=DR,
                            )
                        for j in range(WBLK):
                            kk = loc + j
                            sp = (blk == NBLK - 1 and j == WBLK - 1)
                            nc.tensor.matmul(
                                vps[m][:],
                                lhsT=ht[:, kk, :, mrow],
                                rhs=wt[:, j, :, 0:512],
                                start=False, stop=sp, perf_mode=DR,
                            )
                            nc.tensor.matmul(
                                vps1[m][:],
                                lhsT=ht[:, kk, :, mrow],
                                rhs=wt[:, j, :, 512:640],
                                start=False, stop=sp, perf_mode=DR,
                            )
                for m in range(4):
                    jj = sc * 4 + m
                    if m % 2 == 0:
                        nc.vector.tensor_scalar_mul(
                            vS[:, 0:4, jj, :],
                            vps[m][:].rearrange("p (h d) -> p h d", d=128),
                            dsc[:, 2, 0:1],
                        )
                        nc.vector.tensor_scalar_mul(
                            vS[:, 4, jj, :], vps1[m][:], dsc[:, 2, 0:1])
                    else:
                        nc.scalar.activation(
                            vS[:, 0:4, jj, :],
                            vps[m][:].rearrange("p (h d) -> p h d", d=128),
                            mybir.ActivationFunctionType.Copy, scale=dsc[:, 2, 0:1])
                        nc.scalar.activation(
                            vS[:, 4, jj, :], vps1[m][:],
                            mybir.ActivationFunctionType.Copy, scale=dsc[:, 2, 0:1])

            # ---------------- phase A: chunk 0, DMA-paced startup ----------
            wt0 = w_pool.tile([128, WBLK, 2, OPC], E4, tag="wt", name="wt_first")
            wstate["tiles"][-1] = wt0
            nc.sync.dma_start(wt0[:, 0:2], wq_d[0, :, 0:2])
            hid_dma(0, 0, 0, 2)
            nc.sync.dma_start(wt0[:, 2:4], wq_d[0, :, 2:4])
            hid_dma(0, 0, 2, 4)
            w_issue()                     # wq1
            hid_dma(0, 0, 4, 8)
            w_issue()                     # wq2
            hid_dma(0, 0, 8, 12)
            w_issue()                     # wq3
            hid_dma(0, 0, 12, 16)
            w_issue()                     # wq4
            hid_dma(0, 0, 16, 20)
            w_issue()                     # wq5
            hid_dma(0, 1, 0, 8)
            w_issue()                     # wq6
            hid_dma(0, 1, 8, 16)
            w_issue()                     # wq7
            hid_dma(0, 1, 16, 20)

            qk_pass(0, wq_d, qT, 0, 0, first=True)
            qk_pass(0, wk_d, kT, 1, NBLK)
            # attention constants, loaded while chunk-0 v pass computes
            nc.sync.dma_start(kb[:], kb_d[:])
            nc.sync.dma_start(qb[:], qb_d[:])
            nc.sync.dma_start(slopes_t[:], slopes_d[:])
            # prefetch next chunk's first hid half
            hid_dma(1, 0, 0, 10)
            v_pass(0, 2 * NBLK)
            hid_dma(1, 0, 10, 20)
            _mark("phaseA_end")

            # ---------------- phase B: chunks 1-3 + attention(c-1) ---------
            for sc in range(1, N_SCHUNK):
                A = sc - 1
                wb = 3 * NBLK * sc
                # q pass; hid half1 of this chunk arrives during blks 0-4
                hid_dma(sc, 1, 0, 10)
                hid_dma(sc, 1, 10, 20)
                qk_pass(sc, wq_d, qT, 0, wb)
                # boundary 1
                if A >= 1:
                    emit_pv(A - 1, HPC - 1)
                emit_scores(A, 0)
                emit_scores(A, 1)
                qk_pass(sc, wk_d, kT, 1, wb + NBLK)
                # boundary 2
                emit_pv(A, 0)
                emit_scores(A, 2)
                emit_pv(A, 1)
                emit_scores(A, 3)
                if sc + 1 < N_SCHUNK:
                    hid_dma(sc + 1, 0, 0, 10)
                v_pass(sc, wb + 2 * NBLK)
                if sc + 1 < N_SCHUNK:
                    hid_dma(sc + 1, 0, 10, 20)
                # boundary 3
                emit_pv(A, 2)
                emit_scores(A, 4)
                emit_pv(A, 3)
            _mark("phaseB_end")

        # ========== phase C: attention(3) + o_proj ==========
        N_NCHK = H // 512  # 10
        with (
            tc.tile_pool(name="wo", bufs=1) as wo_pool,
            tc.tile_pool(name="oev", bufs=3) as oev_pool,
        ):
            wo_res = wo_pool.tile([128, N_NCHK, 6, 2, 512], E4, tag="wot")

            def wo_dma(nk):
                nc.sync.dma_start(wo_res[:, nk], wo_d[nk])

            def emit_oproj(Cp, nks):
                for nk in nks:
                    wot = wo_res[:, nk]
                    for st in range(4 * Cp, 4 * Cp + 4):
                        stblk = slice(st * 128, (st + 1) * 128)
                        psf = psU.tile([128, 512], F32, tag="u", name="ps_f")
                        for i in range(2):
                            nc.tensor.matmul(
                                psf[:],
                                lhsT=ctxc[:, 2 * i:2 * i + 2, 0, stblk],
                                rhs=wot[:, 2 * i:2 * i + 2, 1, :],
                                start=(i == 0), stop=False, perf_mode=DR,
                            )
                        nc.tensor.matmul(
                            psf[:], lhsT=ctxc[:, 4, :, stblk], rhs=wot[:, 5, :, :],
                            start=False, stop=False, perf_mode=DR,
                        )
                        for h in range(HPC):
                            nc.tensor.matmul(
                                psf[:], lhsT=ctxc[:, h, :, stblk], rhs=wot[:, h, :, :],
                                start=False, stop=(h == HPC - 1), perf_mode=DR,
                            )
                        oe = oev_pool.tile([128, 512], F16, tag="oe")
                        if st % 2 == 0:
                            nc.scalar.activation(
                                oe[:], psf[:], mybir.ActivationFunctionType.Copy,
                                scale=dsc[:, 3, 0:1])
                        else:
                            nc.vector.tensor_scalar_mul(
                                oe[:], psf[:], dsc[:, 3, 0:1])
                        nc.sync.dma_start(
                            out_d[st * 128:(st + 1) * 128,
                                  nk * 512:(nk + 1) * 512], oe[:]
                        )

            wo_dma(0)
            wo_dma(1)
            emit_pv(2, HPC - 1)
            wo_dma(2)
            emit_scores(3, 0)
            wo_dma(3)
            emit_scores(3, 1)
            wo_dma(4)
            emit_pv(3, 0)
            emit_oproj(0, [0, 1])
            wo_dma(5)
            emit_scores(3, 2)
            emit_pv(3, 1)
            emit_oproj(0, [2, 3])
            wo_dma(6)
            emit_scores(3, 3)
            emit_pv(3, 2)
            emit_oproj(0, [4, 5])
            wo_dma(7)
            emit_pv(3, 3)
            emit_scores(3, 4)
            emit_oproj(0, [6, 7])
            wo_dma(8)
            wo_dma(9)
            emit_pv(3, 4)
            emit_oproj(0, [8, 9])
            for Cp in range(1, N_SCHUNK):
                emit_oproj(Cp, range(N_NCHK))

        _mark("phaseC_end")
    nc.compile()
    nc._phase_marks = marks
    return nc


_NC_CACHE = None


def _get_nc():
    global _NC_CACHE
    if _NC_CACHE is None:
        _NC_CACHE = build_nc()
    return _NC_CACHE


def _pow2_scale(x, target=100.0):
    amax = float(np.abs(x).max())
    if amax == 0.0:
        return 1.0
    return 2.0 ** math.floor(math.log2(target / amax))


def _hilo(xs):
    """xs already scaled; returns (hi, lo) e4m3 arrays."""
    hi = xs.astype(NP_E4)
    lo = (xs - hi.astype(np.float32)).astype(NP_E4)
    return hi, lo


def _prep_inputs(hidden_states, w_pack, w_o):
    hs = np.asarray(hidden_states, np.float32).reshape(S, H)
    w_pack = np.asarray(w_pack, np.float32)
    w_o = np.asarray(w_o, np.float32)

    scale = 1.0 / math.sqrt(HD)
    pos = np.arange(S, dtype=np.float32)
    kb = np.ascontiguousarray(np.broadcast_to(pos[None, :], (128, S)).astype(np.float16))

    # hidden: shared across cores
    C_hs = _pow2_scale(hs)
    hh_hi, hh_lo = _hilo(hs * C_hs)
    # hid[sc, p, k, i, s]
    hid = np.empty((N_SCHUNK, 128, NK, 2, S_CHUNK), NP_E4)
    for i, arr in enumerate((hh_hi, hh_lo)):
        # arr[s, hin] -> [sc, s', k, p] -> transpose to [sc, p, k, s']
        a = arr.reshape(N_SCHUNK, S_CHUNK, NK, 128).transpose(0, 3, 2, 1)
        hid[:, :, :, i, :] = a
    hid = np.ascontiguousarray(hid)

    wp = w_pack.reshape(3, NH, HD, H)  # [qkv, head, d, h_in]

    in_maps = []
    for c in range(NCORES):
        heads = [int(_ORDER[8 * hh + c]) for hh in range(HPC)]
        slopes_c = _SLOPES[heads].astype(np.float32)

        dscv = np.zeros((4, HPC), np.float32)

        def wquant(block, row_scale=None, per_tensor=False):
            """Returns [NBLK, 128, WBLK, 2, OPC] e4m3 + per-head descales."""
            wmat = np.ascontiguousarray(
                wp[block][heads].reshape(OPC, H))   # [640 out, 5120 in]
            if row_scale is not None:
                wmat = wmat * row_scale[:, None]
            wT = wmat.T  # [5120 in, 640 out]
            outarr = np.empty((NBLK, 128, WBLK, 2, OPC), NP_E4)
            Cs = np.empty(HPC, np.float32)
            Ct = _pow2_scale(wT) if per_tensor else None
            for oh in range(HPC):
                sub = wT[:, oh * 128:(oh + 1) * 128]
                Cw = Ct if per_tensor else _pow2_scale(sub)
                Cs[oh] = Cw
                hi, lo = _hilo(sub * Cw)
                # [in, 128] -> [blk, j, p, 128]
                hi = hi.reshape(NBLK, WBLK, 128, 128)
                lo = lo.reshape(NBLK, WBLK, 128, 128)
                outarr[:, :, :, 0, oh * 128:(oh + 1) * 128] = lo.transpose(0, 2, 1, 3)
                outarr[:, :, :, 1, oh * 128:(oh + 1) * 128] = hi.transpose(0, 2, 1, 3)
            return np.ascontiguousarray(outarr), Cs

        q_row_scale = np.repeat(scale / slopes_c, HD)   # [640]
        wq_arr, Cq = wquant(0, q_row_scale)
        wk_arr, Ck = wquant(1)
        wv_arr, Cv = wquant(2, per_tensor=True)
        dscv[0] = 1.0 / (C_hs * Cq)
        dscv[1] = 1.0 / (C_hs * Ck)
        dscv[2] = 1.0 / (C_hs * Cv)

        # o_proj: rhs[d, o] = w_o.T[core 640 rows, :]
        wo_rows = np.concatenate([np.arange(h * HD, (h + 1) * HD) for h in heads])
        woT = np.ascontiguousarray(w_o.T[wo_rows])     # [640, 5120]
        C_wo = _pow2_scale(woT)
        wo_hi, wo_lo = _hilo(woT * C_wo)               # [640, 5120]
        wo_arr = np.empty((H // 512, 128, 6, 2, 512), NP_E4)
        for h in range(HPC):
            hi = wo_hi[h * 128:(h + 1) * 128]          # [128, 5120]
            lo = wo_lo[h * 128:(h + 1) * 128]
            wo_arr[:, :, h, 0, :] = lo.reshape(128, H // 512, 512).transpose(1, 0, 2)
            wo_arr[:, :, h, 1, :] = hi.reshape(128, H // 512, 512).transpose(1, 0, 2)
        # special slot 5: d-tile 4 in (hi, lo) order
        wo_arr[:, :, 5, 0, :] = wo_arr[:, :, 4, 1, :]
        wo_arr[:, :, 5, 1, :] = wo_arr[:, :, 4, 0, :]
        wo_arr = np.ascontiguousarray(wo_arr)
        dscv[3, 0] = 1.0 / (C_CTX * C_wo)

        slopes_tile = np.ascontiguousarray(
            np.broadcast_to(slopes_c[None, :], (128, HPC)).astype(np.float32))
        # qb[p, hh, t] = -slope_hh * (128*t + p)
        qb = -(slopes_c[None, :, None] *
               (128.0 * np.arange(N_ST, dtype=np.float32)[None, None, :]
                + np.arange(128, dtype=np.float32)[:, None, None]))
        qb = np.ascontiguousarray(qb.astype(np.float32))
        dsc_tile = np.ascontiguousarray(
            np.broadcast_to(dscv[None, :, :], (128, 4, HPC)).astype(np.float32))

        in_maps.append({
            "hid": hid,
            "wq": wq_arr,
            "wk": wk_arr,
            "wv": wv_arr,
            "wo": wo_arr,
            "kb": kb,
            "qb": qb,
            "slopes": slopes_tile,
            "dsc": dsc_tile,
        })
    return in_maps


def kernel(hidden_states, w_pack, w_o, _trace=False):
    nc = _get_nc()
    in_maps = _prep_inputs(hidden_states, w_pack, w_o)
    res = run_bass_kernel_spmd(nc, in_maps, core_ids=list(range(NCORES)), trace=_trace)
    acc = np.zeros((S, H), np.float64)
    for r in res.results:
        acc += r["out"].astype(np.float64)
    out = acc.astype(np.float32).reshape(1, S, H)
    if _trace:
        return out, res
    return out


# revision 19
# speedup vs baseline: 1.0299x; 1.0299x over previous
"""Baichuan attention (ALiBi + causal) on 8 TRN2 NeuronCores.

Sharding: tensor-parallel over heads, 5 per core, dealt by ALiBi-slope rank
(core c takes slope-ranks {8*slot + c}) so that every core's head-slot hh has
a similar attention span; o_proj partials are summed on host.

Precision: compensated fp8 (hi+lo e4m3, per-tensor/per-head pow2 scaling) with
DoubleRow perf mode for the QKV projection and o_proj; fp16 for QK/PV;
ALiBi-decay tile skipping per head-slot.

Schedule: software-pipelined across chunks —
  phase A: QKV(chunk 0) with DMA-paced startup
  phase B: QKV(chunk c) with attention(chunk c-1) interleaved at pass
           boundaries (softmax chains hide behind dense QKV matmuls)
  phase C: attention(chunk 3) interleaved with o_proj(chunk 0), then
           o_proj(chunks 1-3); o_proj weights streamed at phase start.

All shapes hardcoded for: B=1, S=2048, H=5120, nh=40, hd=128.
"""

import math
from contextlib import ExitStack

import numpy as np
import ml_dtypes

import concourse.bass as bass
import concourse.bacc as bacc
import concourse.mybir as mybir
import concourse.tile as tile
from concourse.bass_utils import run_bass_kernel_spmd

E4 = mybir.dt.float8e4
F16 = mybir.dt.float16
F32 = mybir.dt.float32
NP_E4 = ml_dtypes.float8_e4m3

NH = 40
HD = 128
H = NH * HD          # 5120
S = 2048
NCORES = 8
HPC = NH // NCORES   # heads per core = 5
OPC = HPC * HD       # output features per core = 640

S_CHUNK = 512
N_SCHUNK = S // S_CHUNK          # 4
NK = H // 128                    # 40 k-tiles (contraction for QKV)
N_ST = S // 128                  # 16 s-tiles
WBLK = 4                         # k-tiles per weight DMA block
NBLK = NK // WBLK                # 10
HID_HALF = NK // 2               # 20 k-tiles per hid half-tile
C_CTX = 16.0                     # fixed on-chip scale for ctx fp8 split
DR = mybir.MatmulPerfMode.DoubleRow
D_CHUNK = 512                    # score-row chunk (1 PSUM bank)
PTC_CAP = 14                     # max sk-tiles per pTc segment
# ALiBi prune: keep sk-tile j for sq-tile t iff slope*(128*(t-j)) < TILE_D.
# Worst-case dropped-mass exponent is TILE_D - slope*127 (first row of the
# query tile), so TILE_D=9 bounds leaked softmax mass at ~e^-7 per head.
TILE_D = 9.0


def _alibi_slopes(n: int):
    def pow2_slopes(k):
        start = 2.0 ** (-(2.0 ** -(math.log2(k) - 3)))
        return [start * (start ** i) for i in range(k)]
    if math.log2(n).is_integer():
        return pow2_slopes(n)
    closest = 2 ** int(math.floor(math.log2(n)))
    return pow2_slopes(closest) + _alibi_slopes(2 * closest)[0::2][: n - closest]


_SLOPES = np.array(_alibi_slopes(NH), np.float64)
# head dealing: rank by slope desc; core c, slot hh -> head _ORDER[8*hh + c]
_ORDER = np.argsort(-_SLOPES, kind="stable")
# per-slot causal+ALiBi tile cap: keep sk-tile j for sq-tile t iff t - j < tcap
_TCAP = []
for hh in range(HPC):
    smin = _SLOPES[_ORDER[8 * hh:8 * hh + 8]].min()
    D = TILE_D / smin
    _TCAP.append(min(N_ST, int(math.floor((D + 127.0) / 128.0)) + 1))


def _jlo(t, hh):
    return max(0, t - _TCAP[hh] + 1)


def build_nc() -> bass.Bass:
    nc = bacc.Bacc(None)
    marks = {}

    def _mark(phase):
        import re as _re
        mx = 0
        for _n in nc.inst_map:
            m = _re.match(r'I-(\d+)$', _n)
            if m:
                mx = max(mx, int(m.group(1)))
        marks[phase] = mx + 1

    # hid[sc, p, k, i, s]: i=0 hi, i=1 lo of C_hs*hidden[sc*512+s, k*128+p]
    hid_d = nc.declare_dram_parameter(
        "hid", [N_SCHUNK, 128, NK, 2, S_CHUNK], E4, isOutput=False)
    # w*[blk, p, j, i, o]: i=0 lo, i=1 hi of scaled w.T[blk*512+j*128+p, o]
    wq_d = nc.declare_dram_parameter("wq", [NBLK, 128, WBLK, 2, OPC], E4, isOutput=False)
    wk_d = nc.declare_dram_parameter("wk", [NBLK, 128, WBLK, 2, OPC], E4, isOutput=False)
    wv_d = nc.declare_dram_parameter("wv", [NBLK, 128, WBLK, 2, OPC], E4, isOutput=False)
    # wo[nk, p, slot, i, o]: slots 0..4 = d-tile h with (lo,hi); slot 5 = d-tile 4 (hi,lo)
    wo_d = nc.declare_dram_parameter("wo", [H // 512, 128, 6, 2, 512], E4, isOutput=False)
    kb_d = nc.declare_dram_parameter("kb", [128, S], F16, isOutput=False)
    qb_d = nc.declare_dram_parameter("qb", [128, HPC, N_ST], F32, isOutput=False)
    slopes_d = nc.declare_dram_parameter("slopes", [128, HPC], F32, isOutput=False)
    # dsc[p, 0..2, hh] = descale for q/k/v psum of head-slot hh; dsc[p, 3, 0] = o_proj descale
    dsc_d = nc.declare_dram_parameter("dsc", [128, 4, HPC], F32, isOutput=False)
    out_d = nc.declare_dram_parameter("out", [S, H], F16, isOutput=True)

    with ExitStack() as ctx:
        tc = ctx.enter_context(tile.TileContext(nc))

        # ---- persistent SBUF residents ----
        qkv_pool = ctx.enter_context(tc.tile_pool(name="qkv", bufs=1))

        qT = qkv_pool.tile([128, HPC, S], F16, tag="qT")     # qT[p, hh, s] = q[s, hh*128+p]
        kT = qkv_pool.tile([128, HPC, S], F16, tag="kT")
        vS = qkv_pool.tile([128, HPC, N_ST, 128], F16, tag="vS")  # vS[p, hh, j, d]
        # ctxc[p, hh, i, s]: i=0 hi, i=1 lo of C_CTX*ctx[s, hh*128+p]
        ctxc = qkv_pool.tile([128, HPC, 2, S], E4, tag="ctxc")
        dsc = qkv_pool.tile([128, 4, HPC], F32, tag="dsc")
        nc.sync.dma_start(dsc[:], dsc_d[:])
        kb = qkv_pool.tile([128, S], F16, tag="kb")
        qb = qkv_pool.tile([128, HPC, N_ST], F32, tag="qb")
        slopes_t = qkv_pool.tile([128, HPC], F32, tag="slopes_t")

        # ---- unified PSUM pool: one rotating ring of 8 banks shared by the
        # QKV passes (5 live), V tail tiles, scores, PV, and o_proj.  PSUM
        # accumulation groups are tracked per 2KB zero region (= bank), so
        # every tile gets its own bank; a single tag keeps allocation FIFO.
        psU = ctx.enter_context(tc.tile_pool(name="psU", bufs=8, space="PSUM"))
        pexp_pool = ctx.enter_context(tc.tile_pool(name="pexp", bufs=4))
        pnorm_pool = ctx.enter_context(tc.tile_pool(name="pnorm", bufs=3))
        ptmp_pool = ctx.enter_context(tc.tile_pool(name="ptmp", bufs=2))
        pT_pool = ctx.enter_context(tc.tile_pool(name="pTc", bufs=2))
        stats_pool = ctx.enter_context(tc.tile_pool(name="stats", bufs=8))

        pTcs = {}

        def emit_scores(C, hh):
            jmin = _jlo(4 * C, hh)             # lowest sk-tile used in chunk
            njc = 4 * C + 4 - jmin
            segs = []                          # (j_abs_start, width, tile)
            for s0 in range(0, njc, PTC_CAP):
                w = min(PTC_CAP, njc - s0)
                ptile = pT_pool.tile([128, w, 512], F16, tag="pTc",
                                     name=f"pTc_{C}_{hh}_{s0}",
                                     padded_shape=[128, PTC_CAP, 512])
                segs.append((jmin + s0, w, ptile))
            pTcs[(C, hh)] = segs

            def ptc_slot(j_abs):
                for js, w, ptile in segs:
                    if js <= j_abs < js + w:
                        return ptile, j_abs - js
                raise AssertionError(j_abs)

            for ti in range(4):
                t = 4 * C + ti
                j0 = _jlo(t, hh)
                L = 128 * (t + 1 - j0)         # score row width (cols j0*128..)
                k_base = j0 * 128
                nch = (L + D_CHUNK - 1) // D_CHUNK
                rs = stats_pool.tile([128, 4], F32, tag="rs")
                pexp_tiles = []
                for ci in range(nch):
                    W = min(D_CHUNK, L - ci * D_CHUNK)
                    ps = psU.tile([128, D_CHUNK], F32, tag="u", name="ps_s")
                    kc0 = k_base + ci * D_CHUNK
                    nc.tensor.matmul(
                        ps[:, 0:W],
                        lhsT=qT[:, hh, t * 128:(t + 1) * 128],
                        rhs=kT[:, hh, kc0:kc0 + W],
                        start=True, stop=True,
                    )
                    # + sk on DVE (the -sq part rides the exp bias); only DVE
                    # and Act can read PSUM, and this add gates psS release
                    tmp = ptmp_pool.tile([128, D_CHUNK], F32, tag="ptmp")
                    nc.vector.tensor_add(tmp[:, :W], ps[:, :W], kb[:, kc0:kc0 + W])
                    if ci == nch - 1:
                        # causal mask on the diagonal 128-block: keep j<=p
                        nc.gpsimd.affine_select(
                            tmp[:, W - 128:W], tmp[:, W - 128:W],
                            pattern=[[-1, 128]],
                            compare_op=mybir.AluOpType.is_ge,
                            fill=-1e9, base=0, channel_multiplier=1)
                    pe = pexp_pool.tile([128, D_CHUNK], F16, tag="pe")
                    nc.scalar.activation(
                        pe[:, :W], tmp[:, :W],
                        mybir.ActivationFunctionType.Exp,
                        scale=slopes_t[:, hh:hh + 1],
                        bias=qb[:, hh, t:t + 1],
                        accum_out=rs[:, ci:ci + 1],
                    )
                    pexp_tiles.append(pe)

                rcp = stats_pool.tile([128, 1], F32, tag="rcp")
                if nch > 1:
                    tot = stats_pool.tile([128, 1], F32, tag="tot")
                    nc.vector.reduce_sum(tot[:], rs[:, :nch], axis=mybir.AxisListType.X)
                    nc.vector.reciprocal(rcp[:], tot[:])
                else:
                    nc.vector.reciprocal(rcp[:], rs[:, 0:1])

                for ci in range(nch):
                    W = min(D_CHUNK, L - ci * D_CHUNK)
                    nb = W // 128
                    pn = pnorm_pool.tile([128, D_CHUNK], F16, tag="pn")
                    nc.vector.tensor_scalar_mul(pn[:, :W], pexp_tiles[ci][:, :W], rcp[:, 0:1])
                    for jj in range(nb):
                        j_abs = j0 + ci * (D_CHUNK // 128) + jj
                        ptile, slot = ptc_slot(j_abs)
                        nc.sync.dma_start_transpose(
                            out=ptile[:, slot, ti * 128:(ti + 1) * 128],
                            in_=pn[:, jj * 128:(jj + 1) * 128],
                        )

        def emit_pv(C, hh):
            segs = pTcs.pop((C, hh))

            def ptc_slot(j_abs):
                for js, w, ptile in segs:
                    if js <= j_abs < js + w:
                        return ptile, j_abs - js
                raise AssertionError(j_abs)

            pso = psU.tile([128, 512], F32, tag="u", name="ps_o")
            for ti in range(4):
                t = 4 * C + ti
                j0 = _jlo(t, hh)
                for j in range(j0, t + 1):
                    ptile, slot = ptc_slot(j)
                    nc.tensor.matmul(
                        pso[:, ti * 128:(ti + 1) * 128],
                        lhsT=vS[:, hh, j, :],
                        rhs=ptile[:, slot, ti * 128:(ti + 1) * 128],
                        start=(j == j0), stop=(j == t),
                    )
            # ctx -> fp8 hi/lo at scale C_CTX
            cblk = slice(C * 512, (C + 1) * 512)
            if (C * HPC + hh) % 2 == 0:
                nc.scalar.activation(
                    ctxc[:, hh, 0, cblk], pso[:],
                    mybir.ActivationFunctionType.Copy, scale=C_CTX)
            else:
                nc.vector.tensor_scalar_mul(
                    ctxc[:, hh, 0, cblk], pso[:], C_CTX)
            nc.vector.scalar_tensor_tensor(
                ctxc[:, hh, 1, cblk], pso[:], C_CTX, ctxc[:, hh, 0, cblk],
                op0=mybir.AluOpType.mult, op1=mybir.AluOpType.subtract)

        # ================= QKV + interleaved attention ======================
        with (
            tc.tile_pool(name="hid", bufs=3) as hid_pool,
            tc.tile_pool(name="wstream", bufs=4) as w_pool,
        ):
            # -- weight stream: global consumption order, prefetch ahead --
            worder = []
            for sc in range(N_SCHUNK):
                for w_d in (wq_d, wk_d, wv_d):
                    for blk in range(NBLK):
                        worder.append((w_d, blk))
            wstate = {"next": 1, "tiles": {}}   # idx 0 (chunk0 wq blk0) is manual

            def w_issue(n=1):
                for _ in range(n):
                    i = wstate["next"]
                    if i >= len(worder):
                        return
                    w_d, blk = worder[i]
                    t = w_pool.tile([128, WBLK, 2, OPC], E4, tag="wt", name=f"wt{i}")
                    nc.sync.dma_start(t[:], w_d[blk])
                    wstate["tiles"][i] = t
                    wstate["next"] = i + 1

            def w_issue_until(i):
                while wstate["next"] <= min(i, len(worder) - 1):
                    w_issue()

            def w_get(i):
                return wstate["tiles"].pop(i)

            # -- hid half-tiles --
            hid_tiles = {}

            def hid_tile(sc, half):
                key = (sc, half)
                if key not in hid_tiles:
                    hid_tiles[key] = hid_pool.tile(
                        [128, HID_HALF, 2, S_CHUNK], E4, tag="hid",
                        name=f"hid_{sc}_{half}")
                return hid_tiles[key]

            def hid_dma(sc, half, k0, k1):
                ht = hid_tile(sc, half)
                nc.sync.dma_start(ht[:, k0:k1],
                                  hid_d[sc, :, half * HID_HALF + k0:half * HID_HALF + k1])

            def ht_slices(sc, blk):
                """(tile, local k offset) for k-tiles blk*4..blk*4+4."""
                half, loc = divmod(blk * WBLK, HID_HALF)
                return hid_tile(sc, half), loc

            def qk_pass(sc, w_d, dest, di, wbase, first=False):
                """q or k pass: psum[oi] = [128 o, 512 s]."""
                pss = [psU.tile([128, S_CHUNK], F32, tag="u", name=f"ps{_i}")
                       for _i in range(HPC)]
                for blk in range(NBLK):
                    if first and blk == 0:
                        wt = wstate["tiles"].pop(-1)   # manual chunk-0 wq blk0
                    else:
                        wt = w_get(wbase + blk)
                    w_issue_until(wbase + blk + 3)
                    ht, loc = ht_slices(sc, blk)
                    for oi in range(HPC):
                        ocol = slice(oi * 128, (oi + 1) * 128)
                        for i in range(WBLK // 2):   # main: hi x hi pairs
                            nc.tensor.matmul(
                                pss[oi][:],
                                lhsT=wt[:, 2 * i:2 * i + 2, 1, ocol],
                                rhs=ht[:, loc + 2 * i:loc + 2 * i + 2, 0, :],
                                start=(blk == 0 and i == 0), stop=False,
                                perf_mode=DR,
                            )
                        for j in range(WBLK):        # cross: (lo,hi) x (hi,lo)
                            nc.tensor.matmul(
                                pss[oi][:],
                                lhsT=wt[:, j, :, ocol],
                                rhs=ht[:, loc + j, :, :],
                                start=False,
                                stop=(blk == NBLK - 1 and j == WBLK - 1),
                                perf_mode=DR,
                            )
                for oi in range(HPC):
                    dslice = dest[:, oi, sc * S_CHUNK:(sc + 1) * S_CHUNK]
                    if oi % 2 == 0:
                        nc.vector.tensor_scalar_mul(
                            dslice, pss[oi][:], dsc[:, di, oi:oi + 1])
                    else:
                        nc.scalar.activation(
                            dslice, pss[oi][:],
                            mybir.ActivationFunctionType.Copy,
                            scale=dsc[:, di, oi:oi + 1])

            def v_pass(sc, wbase):
                """v pass: per m-tile psum [128 s, 512 o] + packed 128-col tile."""
                vps = [psU.tile([128, S_CHUNK], F32, tag="u", name=f"vps0_{m}")
                       for m in range(4)]
                vps1 = [psU.tile([128, 128], F32, tag="u", name=f"vps1_{m}",
                                 padded_shape=[128, 512])
                        for m in range(4)]
                for blk in range(NBLK):
                    wt = w_get(wbase + blk)
                    w_issue_until(wbase + blk + 3)
                    ht, loc = ht_slices(sc, blk)
                    for m in range(4):
                        mrow = slice(m * 128, (m + 1) * 128)
                        for i in range(WBLK // 2):
                            kk = loc + 2 * i
                            st = (blk == 0 and i == 0)
                            nc.tensor.matmul(
                                vps[m][:],
                                lhsT=ht[:, kk:kk + 2, 0, mrow],
                                rhs=wt[:, 2 * i:2 * i + 2, 1, 0:512],
                                start=st, stop=False, perf_mode=DR,
                            )
                            nc.tensor.matmul(
                                vps1[m][:],
                                lhsT=ht[:, kk:kk + 2, 0, mrow],
                                rhs=wt[:, 2 * i:2 * i + 2, 1, 512:640],
                                start=st, stop=False, perf_mode=DR,
                            )
                        for j in range(WBLK):
                            kk = loc + j
                            sp = (blk == NBLK - 1 and j == WBLK - 1)
                            nc.tensor.matmul(
                                vps[m][:],
                                lhsT=ht[:, kk, :, mrow],
                                rhs=wt[:, j, :, 0:512],
                                start=False, stop=sp, perf_mode=DR,
                            )
                            nc.tensor.matmul(
                                vps1[m][:],
                                lhsT=ht[:, kk, :, mrow],
                                rhs=wt[:, j, :, 512:640],
                                start=False, stop=sp, perf_mode=DR,
                            )
                for m in range(4):
                    jj = sc * 4 + m
                    if m % 2 == 0:
                        nc.vector.tensor_scalar_mul(
                            vS[:, 0:4, jj, :],
                            vps[m][:].rearrange("p (h d) -> p h d", d=128),
                            dsc[:, 2, 0:1],
                        )
                        nc.vector.tensor_scalar_mul(
                            vS[:, 4, jj, :], vps1[m][:], dsc[:, 2, 0:1])
                    else:
                        nc.scalar.activation(
                            vS[:, 0:4, jj, :],
                            vps[m][:].rearrange("p (h d) -> p h d", d=128),
                            mybir.ActivationFunctionType.Copy, scale=dsc[:, 2, 0:1])
                        nc.scalar.activation(
                            vS[:, 4, jj, :], vps1[m][:],
                            mybir.ActivationFunctionType.Copy, scale=dsc[:, 2, 0:1])

            # ---------------- phase A: chunk 0, DMA-paced startup ----------
            wt0 = w_pool.tile([128, WBLK, 2, OPC], E4, tag="wt", name="wt_first")
            wstate["tiles"][-1] = wt0
            nc.sync.dma_start(wt0[:, 0:2], wq_d[0, :, 0:2])
            hid_dma(0, 0, 0, 2)
            nc.sync.dma_start(wt0[:, 2:4], wq_d[0, :, 2:4])
            hid_dma(0, 0, 2, 4)
            w_issue()                     # wq1
            hid_dma(0, 0, 4, 8)
            w_issue()                     # wq2
            hid_dma(0, 0, 8, 12)
            w_issue()                     # wq3
            hid_dma(0, 0, 12, 16)
            w_issue()                     # wq4
            hid_dma(0, 0, 16, 20)
            w_issue()                     # wq5
            hid_dma(0, 1, 0, 8)
            w_issue()                     # wq6
            hid_dma(0, 1, 8, 16)
            w_issue()                     # wq7
            hid_dma(0, 1, 16, 20)

            qk_pass(0, wq_d, qT, 0, 0, first=True)
            qk_pass(0, wk_d, kT, 1, NBLK)
            # attention constants, loaded while chunk-0 v pass computes
            nc.sync.dma_start(kb[:], kb_d[:])
            nc.sync.dma_start(qb[:], qb_d[:])
            nc.sync.dma_start(slopes_t[:], slopes_d[:])
            # prefetch next chunk's first hid half
            hid_dma(1, 0, 0, 10)
            v_pass(0, 2 * NBLK)
            hid_dma(1, 0, 10, 20)
            _mark("phaseA_end")

            # ---------------- phase B: chunks 1-3 + attention(c-1) ---------
            for sc in range(1, N_SCHUNK):
                A = sc - 1
                wb = 3 * NBLK * sc
                # q pass; hid half1 of this chunk arrives during blks 0-4
                hid_dma(sc, 1, 0, 10)
                hid_dma(sc, 1, 10, 20)
                qk_pass(sc, wq_d, qT, 0, wb)
                # boundary 1
                if A >= 1:
                    emit_pv(A - 1, HPC - 1)
                emit_scores(A, 0)
                emit_scores(A, 1)
                qk_pass(sc, wk_d, kT, 1, wb + NBLK)
                # boundary 2
                emit_pv(A, 0)
                emit_scores(A, 2)
                emit_pv(A, 1)
                emit_scores(A, 3)
                if sc + 1 < N_SCHUNK:
                    hid_dma(sc + 1, 0, 0, 10)
                else:
                    # last chunk: pull boundary 3 ahead of the v pass (it only
                    # touches chunks <= A) so the post-v-pass slot can start
                    # chunk-3 scores, whose chains resolve before phase C
                    emit_pv(A, 2)
                    emit_scores(A, 4)
                    emit_pv(A, 3)
                v_pass(sc, wb + 2 * NBLK)
                if sc + 1 < N_SCHUNK:
                    hid_dma(sc + 1, 0, 10, 20)
                    # boundary 3
                    emit_pv(A, 2)
                    emit_scores(A, 4)
                    emit_pv(A, 3)
                else:
                    emit_scores(3, 0)
                    emit_pv(A, 4)
                    emit_scores(3, 1)
            _mark("phaseB_end")

        # ========== phase C: attention(3) + o_proj ==========
        N_NCHK = H // 512  # 10
        with (
            tc.tile_pool(name="wo", bufs=1) as wo_pool,
            tc.tile_pool(name="oev", bufs=6) as oev_pool,
        ):
            wo_res = wo_pool.tile([128, N_NCHK, 6, 2, 512], E4, tag="wot")

            def wo_dma(nk):
                nc.sync.dma_start(wo_res[:, nk], wo_d[nk])

            def emit_oproj(Cp, nks):
                for nk in nks:
                    wot = wo_res[:, nk]
                    for st in range(4 * Cp, 4 * Cp + 4):
                        stblk = slice(st * 128, (st + 1) * 128)
                        psf = psU.tile([128, 512], F32, tag="u", name="ps_f")
                        for i in range(2):
                            nc.tensor.matmul(
                                psf[:],
                                lhsT=ctxc[:, 2 * i:2 * i + 2, 0, stblk],
                                rhs=wot[:, 2 * i:2 * i + 2, 1, :],
                                start=(i == 0), stop=False, perf_mode=DR,
                            )
                        nc.tensor.matmul(
                            psf[:], lhsT=ctxc[:, 4, :, stblk], rhs=wot[:, 5, :, :],
                            start=False, stop=False, perf_mode=DR,
                        )
                        for h in range(HPC):
                            nc.tensor.matmul(
                                psf[:], lhsT=ctxc[:, h, :, stblk], rhs=wot[:, h, :, :],
                                start=False, stop=(h == HPC - 1), perf_mode=DR,
                            )
                        oe = oev_pool.tile([128, 512], F16, tag="oe")
                        if st % 2 == 0:
                            nc.scalar.activation(
                                oe[:], psf[:], mybir.ActivationFunctionType.Copy,
                                scale=dsc[:, 3, 0:1])
                        else:
                            nc.vector.tensor_scalar_mul(
                                oe[:], psf[:], dsc[:, 3, 0:1])
                        nc.sync.dma_start(
                            out_d[st * 128:(st + 1) * 128,
                                  nk * 512:(nk + 1) * 512], oe[:]
                        )

            wo_dma(0)
            wo_dma(1)
            emit_pv(3, 0)
            wo_dma(2)
            wo_dma(3)
            emit_scores(3, 2)
            emit_pv(3, 1)
            emit_oproj(0, [0, 1])
            wo_dma(4)
            wo_dma(5)
            emit_scores(3, 3)
            emit_pv(3, 2)
            emit_oproj(0, [2, 3])
            wo_dma(6)
            wo_dma(7)
            emit_pv(3, 3)
            emit_scores(3, 4)
            emit_oproj(0, [4, 5])
            wo_dma(8)
            wo_dma(9)
            emit_oproj(0, [6, 7])
            emit_pv(3, 4)
            emit_oproj(0, [8, 9])
            for Cp in range(1, N_SCHUNK):
                emit_oproj(Cp, range(N_NCHK))

        _mark("phaseC_end")
    nc.compile()
    nc._phase_marks = marks
    return nc


_NC_CACHE = None


def _get_nc():
    global _NC_CACHE
    if _NC_CACHE is None:
        _NC_CACHE = build_nc()
    return _NC_CACHE


def _pow2_scale(x, target=100.0):
    amax = float(np.abs(x).max())
    if amax == 0.0:
        return 1.0
    return 2.0 ** math.floor(math.log2(target / amax))


def _hilo(xs):
    """xs already scaled; returns (hi, lo) e4m3 arrays."""
    hi = xs.astype(NP_E4)
    lo = (xs - hi.astype(np.float32)).astype(NP_E4)
    return hi, lo


def _prep_inputs(hidden_states, w_pack, w_o):
    hs = np.asarray(hidden_states, np.float32).reshape(S, H)
    w_pack = np.asarray(w_pack, np.float32)
    w_o = np.asarray(w_o, np.float32)

    scale = 1.0 / math.sqrt(HD)
    pos = np.arange(S, dtype=np.float32)
    kb = np.ascontiguousarray(np.broadcast_to(pos[None, :], (128, S)).astype(np.float16))

    # hidden: shared across cores
    C_hs = _pow2_scale(hs)
    hh_hi, hh_lo = _hilo(hs * C_hs)
    # hid[sc, p, k, i, s]
    hid = np.empty((N_SCHUNK, 128, NK, 2, S_CHUNK), NP_E4)
    for i, arr in enumerate((hh_hi, hh_lo)):
        # arr[s, hin] -> [sc, s', k, p] -> transpose to [sc, p, k, s']
        a = arr.reshape(N_SCHUNK, S_CHUNK, NK, 128).transpose(0, 3, 2, 1)
        hid[:, :, :, i, :] = a
    hid = np.ascontiguousarray(hid)

    wp = w_pack.reshape(3, NH, HD, H)  # [qkv, head, d, h_in]

    in_maps = []
    for c in range(NCORES):
        heads = [int(_ORDER[8 * hh + c]) for hh in range(HPC)]
        slopes_c = _SLOPES[heads].astype(np.float32)

        dscv = np.zeros((4, HPC), np.float32)

        def wquant(block, row_scale=None, per_tensor=False):
            """Returns [NBLK, 128, WBLK, 2, OPC] e4m3 + per-head descales."""
            wmat = np.ascontiguousarray(
                wp[block][heads].reshape(OPC, H))   # [640 out, 5120 in]
            if row_scale is not None:
                wmat = wmat * row_scale[:, None]
            wT = wmat.T  # [5120 in, 640 out]
            outarr = np.empty((NBLK, 128, WBLK, 2, OPC), NP_E4)
            Cs = np.empty(HPC, np.float32)
            Ct = _pow2_scale(wT) if per_tensor else None
            for oh in range(HPC):
                sub = wT[:, oh * 128:(oh + 1) * 128]
                Cw = Ct if per_tensor else _pow2_scale(sub)
                Cs[oh] = Cw
                hi, lo = _hilo(sub * Cw)
                # [in, 128] -> [blk, j, p, 128]
                hi = hi.reshape(NBLK, WBLK, 128, 128)
                lo = lo.reshape(NBLK, WBLK, 128, 128)
                outarr[:, :, :, 0, oh * 128:(oh + 1) * 128] = lo.transpose(0, 2, 1, 3)
                outarr[:, :, :, 1, oh * 128:(oh + 1) * 128] = hi.transpose(0, 2, 1, 3)
            return np.ascontiguousarray(outarr), Cs

        q_row_scale = np.repeat(scale / slopes_c, HD)   # [640]
        wq_arr, Cq = wquant(0, q_row_scale)
        wk_arr, Ck = wquant(1)
        wv_arr, Cv = wquant(2, per_tensor=True)
        dscv[0] = 1.0 / (C_hs * Cq)
        dscv[1] = 1.0 / (C_hs * Ck)
        dscv[2] = 1.0 / (C_hs * Cv)

        # o_proj: rhs[d, o] = w_o.T[core 640 rows, :]
        wo_rows = np.concatenate([np.arange(h * HD, (h + 1) * HD) for h in heads])
        woT = np.ascontiguousarray(w_o.T[wo_rows])     # [640, 5120]
        C_wo = _pow2_scale(woT)
        wo_hi, wo_lo = _hilo(woT * C_wo)               # [640, 5120]
        wo_arr = np.empty((H // 512, 128, 6, 2, 512), NP_E4)
        for h in range(HPC):
            hi = wo_hi[h * 128:(h + 1) * 128]          # [128, 5120]
            lo = wo_lo[h * 128:(h + 1) * 128]
            wo_arr[:, :, h, 0, :] = lo.reshape(128, H // 512, 512).transpose(1, 0, 2)
            wo_arr[:, :, h, 1, :] = hi.reshape(128, H // 512, 512).transpose(1, 0, 2)
        # special slot 5: d-tile 4 in (hi, lo) order
        wo_arr[:, :, 5, 0, :] = wo_arr[:, :, 4, 1, :]
        wo_arr[:, :, 5, 1, :] = wo_arr[:, :, 4, 0, :]
        wo_arr = np.ascontiguousarray(wo_arr)
        dscv[3, 0] = 1.0 / (C_CTX * C_wo)

        slopes_tile = np.ascontiguousarray(
            np.broadcast_to(slopes_c[None, :], (128, HPC)).astype(np.float32))
        # qb[p, hh, t] = -slope_hh * (128*t + p)
        qb = -(slopes_c[None, :, None] *
               (128.0 * np.arange(N_ST, dtype=np.float32)[None, None, :]
                + np.arange(128, dtype=np.float32)[:, None, None]))
        qb = np.ascontiguousarray(qb.astype(np.float32))
        dsc_tile = np.ascontiguousarray(
            np.broadcast_to(dscv[None, :, :], (128, 4, HPC)).astype(np.float32))

        in_maps.append({
            "hid": hid,
            "wq": wq_arr,
            "wk": wk_arr,
            "wv": wv_arr,
            "wo": wo_arr,
            "kb": kb,
            "qb": qb,
            "slopes": slopes_tile,
            "dsc": dsc_tile,
        })
    return in_maps


def kernel(hidden_states, w_pack, w_o, _trace=False):
    nc = _get_nc()
    in_maps = _prep_inputs(hidden_states, w_pack, w_o)
    res = run_bass_kernel_spmd(nc, in_maps, core_ids=list(range(NCORES)), trace=_trace)
    acc = np.zeros((S, H), np.float64)
    for r in res.results:
        acc += r["out"].astype(np.float64)
    out = acc.astype(np.float32).reshape(1, S, H)
    if _trace:
        return out, res
    return out
